# revision 18
# baseline (speedup 1.0000x reference)
"""Trainium2 Bass kernel for nn_Attention_fusion (sparse_attention fusion block).

Self-contained: takes FULL inputs (B=8 batches), shards batch across 8
NeuronCores (pure data parallel), runs a single fused Bass/Tile kernel per
core, and gathers the full [8,128,128,128] output. BatchNorm batch statistics
are combined across cores with an on-device AllReduce of per-channel moment
partial sums.

v2: pipelined schedule. Phase A (spatial-weight chain + the stats-independent
part of zc) overlaps the input DMA; phase B fuses attention-term/LN/merge/
depthwise-conv/BN-moments into one per-tile loop; engine assignments balance
PE / Act / DVE; c' is spilled to DRAM instead of recomputed; e1 is stored
fp8 (x16 scaling); final affine via diag matmuls + Act bias.
"""
import sys

sys.path.insert(0, "/opt/trn_rl_repo")

import numpy as np

import concourse.bass as bass
import concourse.tile as tile
from concourse import mybir
from concourse.bass_utils import run_bass_kernel_spmd

B, C, H, W = 8, 128, 128, 128
N = H * W
HEADS, HD = 8, 16
EPS_BN = 1e-5
EPS_LN = 1e-5
NCORES = 8
TS = 512                    # free-dim tile size
NT = N // TS                # 32 tiles
PW = W + 2                  # padded row stride (130)
PN = PW * (H + 2)           # padded plane (130*130)
NCH = 8                     # x DMA chunks per input
CHC = N // NCH              # 2048 cols per chunk
DW_SCALE = 16.0             # fp8 scaling for depthwise weights

F32 = mybir.dt.float32
F16 = mybir.dt.float16
F8 = mybir.dt.float8e4
USE_FP8_E1 = False
AX = mybir.AxisListType
ALU = mybir.AluOpType
ACT = mybir.ActivationFunctionType


# ----------------------------------------------------------------------------
# walrus workaround: this container's walrus rejects instructions with more
# than one sync wait command; split extra waits onto standalone EventSemaphore
# instructions on the same engine (program order preserves semantics).
def _split_sync_waits(nc, maxw=1):
    cnt = 0
    for f in nc.m.functions:
        for b in f.blocks:
            insts = b.instructions
            out = []
            changed = False
            for inst in insts:
                si = inst.sync_info
                waits = list(si.on_wait) if si and si.on_wait else []
                if len(waits) > maxw:
                    keep = waits[-maxw:] if maxw > 0 else []
                    extra = waits[: len(waits) - maxw]
                    for wz in extra:
                        es = mybir.InstEventSemaphore(
                            name=f"WSPLIT-{cnt}", ins=[], outs=[]
                        )
                        cnt += 1
                        es.engine = inst.engine
                        es.sync_info = mybir.SyncInfo(on_wait=[wz], on_update=[])
                        out.append(es)
                    inst.sync_info = mybir.SyncInfo(
                        on_wait=keep,
                        on_update=list(si.on_update) if si.on_update else [],
                    )
                    changed = True
                out.append(inst)
            if changed:
                del insts[:]
                insts.extend(out)
    return cnt


# ----------------------------------------------------------------------------
# Host-side weight preparation (identical for every core). All folds:
#  - LN gamma/beta folded into downstream conv weights / biases
#  - mean-centering matrix Cm = I - 11^T/128 folded into the ep weights
#  - attention scale folded into Wk
def _prep_weights(p):
    f32 = lambda a: np.ascontiguousarray(a, np.float32)
    f16 = lambda a: np.ascontiguousarray(a, np.float16)
    w = {}
    J = np.full((C, C), 1.0 / C, np.float64)
    Cm = np.eye(C) - J

    # channel-weights MLP:  y[4C] -> relu(W1 y + b1) -> sigmoid(W2 h + b2)
    W1 = p["cw_w1"].astype(np.float64)      # [512,512]
    W2 = p["cw_w2"].astype(np.float64)      # [256,512]
    w["cw_w1T"] = f16(np.stack([W1[:, k * 128:(k + 1) * 128].T for k in range(4)], 1))
    w["cw_b1"] = f32(p["cw_b1"].reshape(4, 128).T)             # [128,4]
    w["cw_w2T"] = f16(np.stack([W2[:, k * 128:(k + 1) * 128].T for k in range(4)], 1))  # [128,4,256]
    w["cw_b2"] = f32(p["cw_b2"].reshape(2, 128).T)             # [128,2]

    # spatial weight
    w["sw_w1T"] = f16(p["sw_w1"].T)                            # [128,128]
    w["sw_b1"] = f32(p["sw_b1"].reshape(C, 1))
    w["sw_w2T_rep"] = f16(np.repeat(p["sw_w2"].reshape(C, 1), C, 1))  # [128,128]
    w["sw_b2"] = f32(np.full((C, 1), p["sw_b2"][0], np.float32))

    # cross-path projections
    w["cp3T"] = f16(p["cp3_w"].T)
    w["cp3_b"] = f32(p["cp3_b"].reshape(C, 1))
    w["cp4T"] = f16(p["cp4_w"].T)
    w["cp4_b"] = f32(p["cp4_b"].reshape(C, 1))

    # attention: ctxpreT_i = scale * (Wv_i G_i Wk_i^T)  [(h,e),(h,d)]
    scale = HD ** -0.5
    for i, kvw in ((1, p["kv1_w"]), (2, p["kv2_w"])):
        kvw = kvw.astype(np.float64)
        Wk, Wv = kvw[:C], kvw[C:]
        w[f"WkT{i}s"] = f16(Wk.T * scale)                      # [c,(h,d)] scaled
        w[f"WvT{i}"] = f16(Wv.T)                               # [c,(h,e)]

    # ep weights (with Cm centering fold): zc = Cm x + (Cm Ws) s1 + (Cm Wa) a1 + Cm b
    for i, (epw, epb) in ((1, (p["ep1_w"], p["ep1_b"])), (2, (p["ep2_w"], p["ep2_b"]))):
        epw = epw.astype(np.float64)
        Ws, Wa = epw[:, :C], epw[:, C:]
        w[f"WsT{i}c"] = f16((Cm @ Ws).T)                       # [128,128]
        w[f"WaT{i}c"] = f16((Cm @ Wa).T)                       # rhs for device lhsT_a build
        w[f"epb{i}c"] = f32((Cm @ epb.astype(np.float64)).reshape(C, 1))  # [C,1] bias
    w["Cm"] = f16(Cm)                                          # lhsT for the x term (sym)
    w["Jdiv"] = f16(J)                                         # lhsT for var bcast (sym)

    # merge consumers with LN gamma/beta fold
    g1, b1 = p["ln1_g"].astype(np.float64), p["ln1_b"].astype(np.float64)
    g2, b2 = p["ln2_g"].astype(np.float64), p["ln2_b"].astype(np.float64)
    Dres = p["ce_res_w"].astype(np.float64)                    # [128,256]
    Dce1 = p["ce1_w"].astype(np.float64)                       # [128,256]
    w["resT1"] = f16((Dres[:, :C] * g1).T)
    w["resT2"] = f16((Dres[:, C:] * g2).T)
    w["res_bias"] = f32((Dres[:, :C] @ b1 + Dres[:, C:] @ b2).reshape(C, 1))
    w["ce1T1"] = f16((Dce1[:, :C] * g1).T)
    w["ce1T2"] = f16((Dce1[:, C:] * g2).T)
    w["ce1_bias"] = f32((Dce1[:, :C] @ b1 + Dce1[:, C:] @ b2
                         + p["ce1_b"].astype(np.float64)).reshape(C, 1))

    # depthwise 3x3 as 9 diagonal matrices [9,128,128] (tap order dy-major),
    # fp8 with DW_SCALE folded in (undone by the relu activation scale)
    dwk = p["ce_dw_w"].reshape(C, 3, 3).astype(np.float64)
    dwd = np.zeros((9, C, C), np.float64)
    for t in range(9):
        dy, dx = divmod(t, 3)
        np.fill_diagonal(dwd[t], dwk[:, dy, dx] * DW_SCALE)
    w["dw_diag"] = np.ascontiguousarray(
        np.transpose(dwd, (1, 0, 2)), np.float32)              # [C, 9, C] (cast below)
    w["dw_b"] = f32(p["ce_dw_b"].reshape(C, 1))

    w["ce2T"] = f16(p["ce2_w"].T)
    w["ce2_b"] = f32(p["ce2_b"].reshape(C, 1))

    # additive attention mask: 0 on diagonal head blocks, -30000 elsewhere
    m = np.full((C, C), -30000.0, np.float32)
    for hh in range(HEADS):
        m[hh * HD:(hh + 1) * HD, hh * HD:(hh + 1) * HD] = 0.0
    w["attn_mask"] = f32(m)

    # BN params
    w["bn1_g"] = f32(p["ce_bn1_g"].reshape(C, 1))
    w["bn1_b"] = f32(p["ce_bn1_b"].reshape(C, 1))
    w["bn2_g"] = f32(p["ce_bn2_g"].reshape(C, 1))
    w["bn2_b"] = f32(p["ce_bn2_b"].reshape(C, 1))
    return w


def _np_f8(a):
    import ml_dtypes
    return np.ascontiguousarray(np.asarray(a, np.float32).astype(ml_dtypes.float8_e4m3fn))


def _weight_specs(w):
    return {k: (list(v.shape), mybir.dt.from_np(v.dtype)) for k, v in w.items()}


# ----------------------------------------------------------------------------
def _build(wspecs, ncores=NCORES, split=True):
    nc = bass.Bass("TRN2", target_bir_lowering=False, debug=False,
                   num_devices=ncores)
    x1d = nc.dram_tensor("x1", [C, N], F32, kind="ExternalInput").ap()
    x2d = nc.dram_tensor("x2", [C, N], F32, kind="ExternalInput").ap()
    outd = nc.dram_tensor("out", [C, N], F32, kind="ExternalOutput").ap()
    wd = {k: nc.dram_tensor(k, shp, dt, kind="ExternalInput").ap()
          for k, (shp, dt) in wspecs.items()}

    with tile.TileContext(nc) as tc:
        _body(nc, tc, x1d, x2d, outd, wd, ncores)
    if split:
        _split_sync_waits(nc)
    return nc


def _act_rsqrt(nc, out, in_, bias_ap):
    """out = rsqrt(in_ + bias) via the reciprocal_sqrt ACT table (bass's
    activation() refuses Rsqrt; accuracy verified end-to-end)."""
    eng = nc.scalar
    ins = [eng.lower_ap(in_), eng.lower_ap(bias_ap),
           mybir.ImmediateValue(dtype=F32, value=1.0),
           mybir.ImmediateValue(dtype=F32, value=0.0)]
    return eng.add_instruction(
        mybir.InstActivation(
            name=nc.get_next_instruction_name(),
            func=ACT.Rsqrt,
            ins=ins,
            outs=[eng.lower_ap(out)],
        ))


def _body(nc, tc, x1d, x2d, outd, wd, ncores):
    from contextlib import ExitStack
    ctx = ExitStack()
    with ctx:
        wpool = ctx.enter_context(tc.tile_pool(name="w", bufs=1))
        small = ctx.enter_context(tc.tile_pool(name="small", bufs=1))
        dram = ctx.enter_context(tc.tile_pool(name="dram", bufs=1, space="DRAM"))
        # long-lived: zc_partial store (also becomes u store)
        pzc_cm = tc.tile_pool(name="pzc", bufs=1)
        pzc = pzc_cm.__enter__()
        # x store (released after phase 1)
        pxs_cm = tc.tile_pool(name="pxs", bufs=1)
        pxs = pxs_cm.__enter__()
        # trans pools for phase A/1
        trA_cm = tc.tile_pool(name="trA", bufs=2)
        trA = trA_cm.__enter__()
        # PSUM pools: AB spans phase A..B; A1 only phase 1
        psAB_cm = tc.tile_pool(name="psAB", bufs=2, space="PSUM")
        psAB = psAB_cm.__enter__()
        psA1_cm = tc.tile_pool(name="psA1", bufs=2, space="PSUM")
        psA1 = psA1_cm.__enter__()

        # ---- load weights ------------------------------------------------
        wt = {}
        for k in wd:
            shp = list(wd[k].shape)
            if len(shp) == 3:
                flat = [shp[0], shp[1] * shp[2]]
                wt[k] = wpool.tile(flat, wd[k].dtype, tag=k, name="w_" + k)
                nc.sync.dma_start(wt[k][:], wd[k].rearrange("a b c -> a (b c)"))
            else:
                wt[k] = wpool.tile(shp, wd[k].dtype, tag=k, name="w_" + k)
                nc.sync.dma_start(wt[k][:], wd[k][:])

        eps_ln = small.tile([C, 1], F32, tag="eps_ln")
        nc.vector.memset(eps_ln[:], EPS_LN)
        eps_bn = small.tile([C, 1], F32, tag="eps_bn")
        nc.vector.memset(eps_bn[:], EPS_BN)
        # stats accumulators
        avgacc = small.tile([C, 2 * NCH], F32, tag="avgacc")
        m1acc = small.tile([C, TS], F16, tag="m1acc")
        m2acc = small.tile([C, TS], F16, tag="m2acc")
        nc.vector.memset(m1acc[:], -60000.0)
        nc.vector.memset(m2acc[:], -60000.0)
        # moment stats (bn_stats 6-slot records per tile)
        rstat = small.tile([C, NT * 6], F32, tag="rstat")
        estat = small.tile([C, NT * 6], F32, tag="estat")
        erstat = small.tile([C, NT * 6], F32, tag="erstat")

        xs = [pxs.tile([C, N], F16, tag="x1s", name="x1s"),
              pxs.tile([C, N], F16, tag="x2s", name="x2s")]
        zcp = [pzc.tile([C, N], F16, tag="zcp1", name="zcp1"),
               pzc.tile([C, N], F16, tag="zcp2", name="zcp2")]
        cboth_d = dram.tile([C, 2 * N], F16)

        # ---- phase A + x load, chunk by chunk ----------------------------
        WsTc = [wt["WsT1c"], wt["WsT2c"]]
        epbc = [wt["epb1c"], wt["epb2c"]]
        macc = [m1acc, m2acc]
        for c_ in range(NCH):
            csl = bass.ts(c_, CHC)
            t0 = c_ * (CHC // TS)
            ntile = CHC // TS
            for i, xd in ((0, x1d), (1, x2d)):
                nc.gpsimd.dma_start(xs[i][:, csl], xd[:, csl])
            # avg stats per chunk (Act copy+accum)
            for i in range(2):
                junk = trA.tile([C, CHC], F16, tag="sjunk", bufs=1)
                nc.scalar.activation(
                    junk[:], xs[i][:, csl], ACT.Copy,
                    accum_out=avgacc[:, i * NCH + c_:i * NCH + c_ + 1])
            # phase A for the tiles of this chunk
            for t in range(t0, t0 + ntile):
                for i in range(2):
                    xt = xs[i][:, bass.ts(t, TS)]
                    # running max (DVE, in place)
                    nc.vector.tensor_tensor(macc[i][:], macc[i][:], xt, op=ALU.max)
                    # h = relu(W1 x + b1)
                    ph = psAB.tile([C, TS], F32, tag="T1")
                    nc.tensor.matmul(ph[:], wt["sw_w1T"][:], xt, start=True, stop=True)
                    h_t = trA.tile([C, TS], F16, tag="h_t")
                    nc.vector.tensor_scalar(out=h_t[:], in0=ph[:],
                                            scalar1=wt["sw_b1"][:], scalar2=0.0,
                                            op0=ALU.add, op1=ALU.max)
                    # sigma = sigmoid(w2^T h + b2) (replicated)
                    pl = psAB.tile([C, TS], F32, tag="T1")
                    nc.tensor.matmul(pl[:], wt["sw_w2T_rep"][:], h_t[:], start=True, stop=True)
                    sg_t = trA.tile([C, TS], F16, tag="sg_t")
                    nc.scalar.activation(sg_t[:], pl[:], ACT.Sigmoid, bias=wt["sw_b2"][:])
                    # s = sigma * x
                    s_t = trA.tile([C, TS], F16, tag="s_t")
                    nc.vector.tensor_tensor(s_t[:], sg_t[:], xt, op=ALU.mult)
                    # zc_partial = Ws s + Cm x (+ epb bias on evac)
                    pz = psAB.tile([C, TS], F32, tag="T2")
                    nc.tensor.matmul(pz[:], WsTc[i][:], s_t[:], start=True, stop=False)
                    nc.tensor.matmul(pz[:], wt["Cm"][:], xt, start=False, stop=True)
                    if i == 0:
                        nc.vector.tensor_scalar(out=zcp[i][:, bass.ts(t, TS)],
                                                in0=pz[:], scalar1=epbc[i][:],
                                                scalar2=None, op0=ALU.add)
                    else:
                        nc.scalar.activation(zcp[i][:, bass.ts(t, TS)], pz[:],
                                             ACT.Identity, bias=epbc[i][:])

        # ---- stats -> channel MLP ---------------------------------------
        stat_y = small.tile([C, 4], F32, tag="stat_y")       # avg1 avg2 mx1 mx2
        for i in range(2):
            nc.vector.tensor_reduce(
                out=stat_y[:, i:i + 1],
                in_=avgacc[:, i * NCH:(i + 1) * NCH],
                axis=AX.X, op=ALU.add)
            mh = small.tile([C, 256], F16, tag=f"mh{i}")
            nc.vector.tensor_tensor(mh[:], macc[i][:, :256], macc[i][:, 256:], op=ALU.max)
            mq = small.tile([C, 128], F16, tag=f"mq{i}")
            nc.vector.tensor_tensor(mq[:], mh[:, :128], mh[:, 128:], op=ALU.max)
            nc.vector.tensor_reduce(out=stat_y[:, 2 + i:3 + i], in_=mq[:],
                                    axis=AX.X, op=ALU.max)
        y16 = small.tile([C, 4], F16, tag="y16")
        nc.vector.tensor_scalar(out=y16[:, 0:2], in0=stat_y[:, 0:2],
                                scalar1=1.0 / N, scalar2=None, op0=ALU.mult)
        nc.vector.tensor_copy(y16[:, 2:4], stat_y[:, 2:4])

        h16 = small.tile([C, 4], F16, tag="h16")
        for oc in range(4):
            ph = psAB.tile([C, TS], F32, tag="T1")
            for kc in range(4):
                nc.tensor.matmul(
                    ph[:, :1], wt["cw_w1T"][:, kc * 512 + oc * 128: kc * 512 + (oc + 1) * 128],
                    y16[:, kc:kc + 1], start=(kc == 0), stop=(kc == 3))
            nc.scalar.activation(h16[:, oc:oc + 1], ph[:, :1], ACT.Relu,
                                 bias=wt["cw_b1"][:, oc:oc + 1])
        cw = small.tile([C, 2], F32, tag="cw")
        for oc in range(2):
            ph = psAB.tile([C, TS], F32, tag="T1")
            for kc in range(4):
                nc.tensor.matmul(
                    ph[:, :1], wt["cw_w2T"][:, kc * 256 + oc * 128: kc * 256 + (oc + 1) * 128],
                    h16[:, kc:kc + 1], start=(kc == 0), stop=(kc == 3))
            nc.scalar.activation(cw[:, oc:oc + 1], ph[:, :1], ACT.Sigmoid,
                                 bias=wt["cw_b2"][:, oc:oc + 1])
        cpTc = []
        for i in range(2):
            cc = small.tile([C, C], F16, tag=f"cpTc_{i}")
            nc.vector.tensor_scalar(out=cc[:], in0=wt[("cp3T", "cp4T")[i]][:],
                                    scalar1=cw[:, i:i + 1], scalar2=None,
                                    op0=ALU.mult)
            cpTc.append(cc)

        # ---- phase 1: cp chain + Gram (c' spilled to DRAM) ---------------
        gp = psA1.tile([C, 2 * C], F32, tag="gram", bufs=1, name="gram")
        cpB = [wt["cp3_b"], wt["cp4_b"]]
        for t in range(NT):
            cpair = trA.tile([C, 2 * TS], F16, tag="cpair", bufs=3)
            for i in range(2):
                pc = psAB.tile([C, TS], F32, tag="T1")
                nc.tensor.matmul(pc[:], cpTc[i][:], xs[i][:, bass.ts(t, TS)],
                                 start=True, stop=True)
                cdst = cpair[:, i * TS:(i + 1) * TS]
                if i == 0:
                    nc.vector.tensor_scalar(out=cdst, in0=pc[:],
                                            scalar1=cpB[i][:], scalar2=0.0,
                                            op0=ALU.add, op1=ALU.max)
                else:
                    nc.scalar.activation(cdst, pc[:], ACT.Relu, bias=cpB[i][:])
            nc.gpsimd.dma_start(cboth_d[:, t * 2 * TS:(t + 1) * 2 * TS], cpair[:])
            for i in range(2):
                pt = psA1.tile([C, TS], F16, tag="pt", bufs=2)
                for j in range(4):
                    nc.tensor.transpose(
                        pt[:, j * 128:(j + 1) * 128],
                        cpair[:, i * TS + j * 128: i * TS + (j + 1) * 128],
                        wt["ident"][:])
                ctt = trA.tile([C, TS], F16, tag=f"ctt{i}")
                if i == 0:
                    nc.vector.tensor_copy(ctt[:], pt[:])
                else:
                    nc.scalar.copy(ctt[:], pt[:])
                for j in range(4):
                    nc.tensor.matmul(gp[:, i * C:(i + 1) * C],
                                     ctt[:, j * 128:(j + 1) * 128],
                                     ctt[:, j * 128:(j + 1) * 128],
                                     start=(t == 0 and j == 0),
                                     stop=(t == NT - 1 and j == 3))

        # ---- attention context smalls ------------------------------------
        bdp = []
        for i in range(2):
            g16 = small.tile([C, C], F16, tag=f"g16_{i}")
            nc.vector.tensor_copy(g16[:], gp[:, i * C:(i + 1) * C])
            pm = psAB.tile([C, TS], F32, tag="T1")
            nc.tensor.matmul(pm[:, :C], g16[:], wt[f"WkT{i+1}s"][:], start=True, stop=True)
            m16 = small.tile([C, C], F16, tag=f"m16_{i}")
            nc.vector.tensor_copy(m16[:], pm[:, :C])
            pc2 = psAB.tile([C, TS], F32, tag="T1")
            nc.tensor.matmul(pc2[:, :C], wt[f"WvT{i+1}"][:], m16[:], start=True, stop=True)
            cm_t = small.tile([C, C], F32, tag=f"cm_{i}")
            nc.vector.tensor_tensor(cm_t[:], pc2[:, :C], wt["attn_mask"][:], op=ALU.add)
            negmx = small.tile([C, 1], F32, tag=f"negmx_{i}")
            nc.vector.tensor_reduce(out=negmx[:], in_=cm_t[:], axis=AX.X,
                                    op=ALU.max, negate=True)
            ex = small.tile([C, C], F32, tag=f"ex_{i}")
            nc.scalar.activation(ex[:], cm_t[:], ACT.Exp, bias=negmx[:])
            sm = small.tile([C, 1], F32, tag=f"sm_{i}")
            nc.vector.tensor_reduce(out=sm[:], in_=ex[:], axis=AX.X, op=ALU.add)
            rs = small.tile([C, 1], F32, tag=f"rs_{i}")
            nc.vector.reciprocal(rs[:], sm[:])
            bd = small.tile([C, C], F16, tag=f"bd_{i}")
            nc.vector.tensor_scalar(out=bd[:], in0=ex[:], scalar1=rs[:],
                                    scalar2=None, op0=ALU.mult)
            bdp.append(bd)
        lhsTa = []
        for i in range(2):
            pa = psAB.tile([C, TS], F32, tag="T1")
            nc.tensor.matmul(pa[:, :C], bdp[1 - i][:], wt[f"WaT{i+1}c"][:],
                             start=True, stop=True)
            a16 = small.tile([C, C], F16, tag=f"a16_{i}")
            nc.vector.tensor_copy(a16[:], pa[:, :C])
            lhsTa.append(a16)

        # ---- close phase A/1 pools, open phase B pools -------------------
        psA1_cm.__exit__(None, None, None)
        trA_cm.__exit__(None, None, None)
        pxs_cm.__exit__(None, None, None)
        psB_cm = tc.tile_pool(name="psB", bufs=2, space="PSUM")
        psB = psB_cm.__enter__()
        pconv_cm = tc.tile_pool(name="pconv", bufs=1)
        pconv = pconv_cm.__enter__()
        trB_cm = tc.tile_pool(name="trB", bufs=2)
        trB = trB_cm.__enter__()

        res_s = pconv.tile([C, N], F16, tag="res_s")
        e_s = pconv.tile([C, N], F16, tag="e_s")
        e1p = pconv.tile([C, PN], F8 if USE_FP8_E1 else F16, tag="e1p")
        nc.vector.memset(e1p[:], 0.0)   # border stays zero
        e1v = e1p[:].rearrange("p (h w) -> p h w", w=PW)

        def dw_block(t):
            """depthwise conv + ce2 + moments for tile t (e1 rows ready)."""
            sl = bass.ts(t, TS)
            h0 = t * 4
            pdw = psB.tile([C, TS], F32, tag="dw")
            for tap in range(9):
                dy, dx = divmod(tap, 3)
                rhs = e1v[:, h0 + dy: h0 + dy + 4, dx: dx + W]
                nc.tensor.matmul(pdw[:], wt["dw_diag"][:, tap * C:(tap + 1) * C],
                                 rhs, start=(tap == 0), stop=(tap == 8))
            e2_t = trB.tile([C, TS], F16, tag="e2_t")
            nc.scalar.activation(e2_t[:], pdw[:], ACT.Relu, bias=wt["dw_b"][:],
                                 scale=1.0 / DW_SCALE)
            pce = psAB.tile([C, TS], F32, tag="T2")
            nc.tensor.matmul(pce[:], wt["ce2T"][:], e2_t[:], start=True, stop=True)
            nc.scalar.activation(e_s[:, sl], pce[:], ACT.Identity,
                                 bias=wt["ce2_b"][:])
            # moments via bn_stats (count/mean/M2 of even+odd halves)
            nc.vector.bn_stats(estat[:, t * 6:(t + 1) * 6], e_s[:, sl])
            j2 = trB.tile([C, TS], F16, tag="mjunk")
            nc.vector.tensor_tensor(j2[:], e_s[:, sl], res_s[:, sl], op=ALU.mult)
            nc.vector.bn_stats(erstat[:, t * 6:(t + 1) * 6], j2[:])

        # ---- phase B: fused per-tile loop --------------------------------
        PF = 3  # c' reload prefetch distance

        def c_reload(t):
            crl = trB.tile([C, 2 * TS], F16, tag="crl", bufs=PF, name="crl")
            nc.gpsimd.dma_start(crl[:], cboth_d[:, t * 2 * TS:(t + 1) * 2 * TS])
            return crl

        crls = {t: c_reload(t) for t in range(min(PF, NT))}
        for t in range(NT):
            sl = bass.ts(t, TS)
            if t + PF < NT:
                crls[t + PF] = c_reload(t + PF)
            crl = crls.pop(t)
            u = []
            for i in range(2):
                # zc = zc_partial + lhsTa c'
                pz = psAB.tile([C, TS], F32, tag="T2")
                nc.tensor.matmul(pz[:], lhsTa[i][:], crl[:, i * TS:(i + 1) * TS],
                                 start=True, stop=True)
                zc_t = trB.tile([C, TS], F16, tag="zc_t", bufs=3)
                nc.vector.tensor_tensor(zc_t[:], pz[:], zcp[i][:, sl], op=ALU.add)
                zq_t = trB.tile([C, TS], F16, tag="zq_t")
                nc.vector.tensor_tensor(zq_t[:], zc_t[:], zc_t[:], op=ALU.mult)
                pv = psAB.tile([C, TS], F32, tag="T1")
                nc.tensor.matmul(pv[:], wt["Jdiv"][:], zq_t[:], start=True, stop=True)
                rstd_t = trB.tile([C, TS], F16, tag="rstd_t")
                _act_rsqrt(nc, rstd_t[:], pv[:], eps_ln[:])
                # u overwrites the zc_partial store
                nc.vector.tensor_tensor(zcp[i][:, sl], zc_t[:], rstd_t[:], op=ALU.mult)
                u.append(zcp[i][:, sl])
            # res
            pr = psAB.tile([C, TS], F32, tag="T2")
            nc.tensor.matmul(pr[:], wt["resT1"][:], u[0], start=True, stop=False)
            nc.tensor.matmul(pr[:], wt["resT2"][:], u[1], start=False, stop=True)
            nc.scalar.activation(res_s[:, sl], pr[:], ACT.Identity,
                                 bias=wt["res_bias"][:])
            nc.vector.bn_stats(rstat[:, t * 6:(t + 1) * 6], res_s[:, sl])
            # e1 (padded, fp8; the DW_SCALE fold is undone by the dw relu scale)
            pe1 = psB.tile([C, TS], F32, tag="e1ps")
            nc.tensor.matmul(pe1[:], wt["ce1T1"][:], u[0], start=True, stop=False)
            nc.tensor.matmul(pe1[:], wt["ce1T2"][:], u[1], start=False, stop=True)
            h0 = t * 4
            nc.scalar.activation(
                e1v[:, h0 + 1: h0 + 5, 1: 1 + W],
                pe1[:].rearrange("p (a b) -> p a b", b=W),
                ACT.Identity, bias=wt["ce1_bias"][:])
            if t >= 1:
                dw_block(t - 1)
        dw_block(NT - 1)

        # ---- cross-core moment allreduce + coefficients ------------------
        parts = small.tile([C, 8], F32, tag="parts")
        nc.vector.memset(parts[:], 0.0)
        HN = TS // 2  # elements per even/odd half

        def agg_stats(stat, sum_col, sq_col):
            """parts[sum_col] = sum(x); parts[sq_col] = sum(x^2) (if not None),
            reconstructed from per-tile bn_stats records."""
            v = stat[:].rearrange("p (t k) -> p t k", k=6)
            wrk = small.tile([C, NT], F32, tag="aggw1", uniquify=True)
            wv = wrk[:].rearrange("p (t k) -> p t k", k=1)
            nc.vector.tensor_tensor(wv[:], v[:, :, 1:2], v[:, :, 4:5], op=ALU.add)
            nc.vector.tensor_reduce(out=parts[:, sum_col:sum_col + 1], in_=wrk[:],
                                    axis=AX.X, op=ALU.add)
            nc.vector.tensor_scalar(out=parts[:, sum_col:sum_col + 1],
                                    in0=parts[:, sum_col:sum_col + 1],
                                    scalar1=float(HN), scalar2=None, op0=ALU.mult)
            if sq_col is None:
                return
            w2 = small.tile([C, NT], F32, tag="aggw2", uniquify=True)
            w2v = w2[:].rearrange("p (t k) -> p t k", k=1)
            w3 = small.tile([C, NT], F32, tag="aggw3", uniquify=True)
            w3v = w3[:].rearrange("p (t k) -> p t k", k=1)
            # w2 = HN*(me^2 + mo^2) + M2e + M2o, per tile
            nc.vector.tensor_tensor(w2v[:], v[:, :, 1:2], v[:, :, 1:2], op=ALU.mult)
            nc.vector.tensor_tensor(w3v[:], v[:, :, 4:5], v[:, :, 4:5], op=ALU.mult)
            nc.vector.tensor_tensor(w2[:], w2[:], w3[:], op=ALU.add)
            nc.vector.tensor_scalar(out=w2[:], in0=w2[:], scalar1=float(HN),
                                    scalar2=None, op0=ALU.mult)
            nc.vector.tensor_tensor(w3v[:], v[:, :, 2:3], v[:, :, 5:6], op=ALU.add)
            nc.vector.tensor_tensor(w2[:], w2[:], w3[:], op=ALU.add)
            nc.vector.tensor_reduce(out=parts[:, sq_col:sq_col + 1], in_=w2[:],
                                    axis=AX.X, op=ALU.add)

        agg_stats(estat, 0, 1)
        agg_stats(rstat, 2, 3)
        agg_stats(erstat, 4, None)
        cin = dram.tile([C, 8], F32)
        cout = dram.tile([C, 8], F32)
        nc.sync.dma_start(cin[:], parts[:])
        nc.gpsimd.collective_compute(
            "AllReduce", ALU.add,
            replica_groups=[list(range(ncores))],
            ins=[cin.opt()], outs=[cout.opt()])
        allr = small.tile([C, 8], F32, tag="allr")
        nc.sync.dma_start(allr[:], cout[:])

        # per-channel BN coefficient math (tiny [128,1] ops)
        NTOT = float(ncores * N)
        cf = small.tile([C, 12], F32, tag="cf")
        nc.vector.tensor_scalar(out=cf[:, 0:1], in0=allr[:, 0:1], scalar1=1.0 / NTOT,
                                scalar2=None, op0=ALU.mult)               # me
        nc.vector.tensor_scalar(out=cf[:, 1:2], in0=allr[:, 1:2], scalar1=1.0 / NTOT,
                                scalar2=None, op0=ALU.mult)               # E e2
        nc.vector.tensor_tensor(cf[:, 2:3], cf[:, 0:1], cf[:, 0:1], op=ALU.mult)
        nc.vector.tensor_tensor(cf[:, 2:3], cf[:, 1:2], cf[:, 2:3], op=ALU.subtract)  # ve
        tmp = small.tile([C, 4], F32, tag="cftmp")
        nc.scalar.activation(tmp[:, 0:1], cf[:, 2:3], ACT.Sqrt, bias=eps_bn[:])
        nc.vector.reciprocal(tmp[:, 1:2], tmp[:, 0:1])                     # rsq_e
        nc.vector.tensor_tensor(cf[:, 3:4], wt["bn1_g"][:], tmp[:, 1:2], op=ALU.mult)  # a1
        nc.vector.tensor_tensor(cf[:, 4:5], cf[:, 0:1], cf[:, 3:4], op=ALU.mult)
        nc.vector.tensor_tensor(cf[:, 4:5], wt["bn1_b"][:], cf[:, 4:5], op=ALU.subtract)  # b1f
        nc.vector.tensor_tensor(cf[:, 5:6], cf[:, 3:4], allr[:, 0:1], op=ALU.mult)
        nc.vector.tensor_tensor(cf[:, 5:6], allr[:, 2:3], cf[:, 5:6], op=ALU.add)
        nc.vector.tensor_scalar(out=cf[:, 5:6], in0=cf[:, 5:6], scalar1=1.0 / NTOT,
                                scalar2=None, op0=ALU.mult)
        nc.vector.tensor_tensor(cf[:, 5:6], cf[:, 5:6], cf[:, 4:5], op=ALU.add)   # ms
        nc.vector.tensor_tensor(tmp[:, 2:3], cf[:, 3:4], cf[:, 3:4], op=ALU.mult)  # a1^2
        nc.vector.tensor_tensor(tmp[:, 3:4], tmp[:, 2:3], allr[:, 1:2], op=ALU.mult)
        nc.vector.tensor_tensor(cf[:, 6:7], cf[:, 3:4], allr[:, 4:5], op=ALU.mult)
        nc.vector.tensor_scalar(out=cf[:, 6:7], in0=cf[:, 6:7], scalar1=2.0,
                                scalar2=None, op0=ALU.mult)
        nc.vector.tensor_tensor(cf[:, 6:7], cf[:, 6:7], tmp[:, 3:4], op=ALU.add)
        nc.vector.tensor_tensor(cf[:, 6:7], cf[:, 6:7], allr[:, 3:4], op=ALU.add)
        nc.vector.tensor_scalar(out=cf[:, 6:7], in0=cf[:, 6:7], scalar1=1.0 / NTOT,
                                scalar2=None, op0=ALU.mult)
        nc.vector.tensor_tensor(tmp[:, 2:3], cf[:, 5:6], cf[:, 4:5], op=ALU.subtract)
        nc.vector.tensor_tensor(tmp[:, 2:3], tmp[:, 2:3], cf[:, 4:5], op=ALU.mult)
        nc.vector.tensor_scalar(out=tmp[:, 2:3], in0=tmp[:, 2:3], scalar1=2.0,
                                scalar2=None, op0=ALU.mult)
        nc.vector.tensor_tensor(cf[:, 6:7], cf[:, 6:7], tmp[:, 2:3], op=ALU.add)
        nc.vector.tensor_tensor(tmp[:, 2:3], cf[:, 4:5], cf[:, 4:5], op=ALU.mult)
        nc.vector.tensor_tensor(cf[:, 6:7], cf[:, 6:7], tmp[:, 2:3], op=ALU.add)  # Es2
        nc.vector.tensor_tensor(tmp[:, 2:3], cf[:, 5:6], cf[:, 5:6], op=ALU.mult)
        nc.vector.tensor_tensor(cf[:, 7:8], cf[:, 6:7], tmp[:, 2:3], op=ALU.subtract)  # vs
        nc.scalar.activation(tmp[:, 0:1], cf[:, 7:8], ACT.Sqrt, bias=eps_bn[:])
        nc.vector.reciprocal(tmp[:, 1:2], tmp[:, 0:1])
        nc.vector.tensor_tensor(cf[:, 8:9], wt["bn2_g"][:], tmp[:, 1:2], op=ALU.mult)  # a2
        nc.vector.tensor_tensor(cf[:, 9:10], cf[:, 8:9], cf[:, 3:4], op=ALU.mult)      # ae
        # c0 = b2 - a2*(ms - b1f)
        nc.vector.tensor_tensor(tmp[:, 2:3], cf[:, 5:6], cf[:, 4:5], op=ALU.subtract)
        nc.vector.tensor_tensor(tmp[:, 2:3], tmp[:, 2:3], cf[:, 8:9], op=ALU.mult)
        nc.vector.tensor_tensor(cf[:, 10:11], wt["bn2_b"][:], tmp[:, 2:3],
                                op=ALU.subtract)
        # diag matrices for the final affine
        dae = small.tile([C, C], F16, tag="dae")
        nc.vector.tensor_scalar(out=dae[:], in0=wt["ident"][:],
                                scalar1=cf[:, 9:10], scalar2=None, op0=ALU.mult)
        da2 = small.tile([C, C], F16, tag="da2")
        nc.vector.tensor_scalar(out=da2[:], in0=wt["ident"][:],
                                scalar1=cf[:, 8:9], scalar2=None, op0=ALU.mult)

        # ---- final affine + store ----------------------------------------
        psB_cm.__exit__(None, None, None)
        psF_cm = tc.tile_pool(name="psF", bufs=4, space="PSUM")
        psF = psF_cm.__enter__()
        for t in range(NT):
            sl = bass.ts(t, TS)
            pf = psF.tile([C, TS], F32, tag="fin")
            nc.tensor.matmul(pf[:], dae[:], e_s[:, sl], start=True, stop=False)
            nc.tensor.matmul(pf[:], da2[:], res_s[:, sl], start=False, stop=True)
            out_t = trB.tile([C, TS], F16, tag="out_t", bufs=3)
            nc.scalar.activation(out_t[:], pf[:], ACT.Identity, bias=cf[:, 10:11])
            nc.gpsimd.dma_start(outd[:, sl], out_t[:])
        psF_cm.__exit__(None, None, None)
        trB_cm.__exit__(None, None, None)
        pconv_cm.__exit__(None, None, None)
        psAB_cm.__exit__(None, None, None)
        pzc_cm.__exit__(None, None, None)


_CACHE = {}


def _get_nc(wspecs_key, wspecs, ncores=NCORES):
    key = (wspecs_key, ncores)
    if key not in _CACHE:
        _CACHE[key] = _build(wspecs, ncores=ncores)
    return _CACHE[key]


def _prep_all(inputs):
    w = _prep_weights(inputs)
    w["dw_diag"] = _np_f8(w["dw_diag"]) if USE_FP8_E1 else \
        np.ascontiguousarray(w["dw_diag"], np.float16)
    w["ident"] = np.eye(C, dtype=np.float16)
    return w


def kernel(**inputs):
    w = _prep_all(inputs)
    wspecs = _weight_specs(w)
    nc = _get_nc("v2", wspecs)
    x1 = np.ascontiguousarray(inputs["x1"], np.float32).reshape(B, C, N)
    x2 = np.ascontiguousarray(inputs["x2"], np.float32).reshape(B, C, N)
    in_maps = [dict(w, x1=x1[b], x2=x2[b]) for b in range(B)]
    res = run_bass_kernel_spmd(nc, in_maps, list(range(NCORES)))
    out = np.stack([res.results[b]["out"].reshape(C, H, W) for b in range(B)])
    return out.astype(np.float32)


def _ensure_ntff_hook():
    """Synthesize antenv.axon_hooks (missing in this image) and register the
    ctypes NTFF profile hook against the axon PJRT .so."""
    import types
    import antenv
    if getattr(antenv, "axon_hooks", None) is not None:
        return
    mod = types.ModuleType("antenv.axon_hooks")
    mod._hook = None
    def set_axon_ntff_profile_hook(h):
        mod._hook = h
    def get_axon_ntff_profile_hook():
        return mod._hook
    mod.set_axon_ntff_profile_hook = set_axon_ntff_profile_hook
    mod.get_axon_ntff_profile_hook = get_axon_ntff_profile_hook
    sys.modules["antenv.axon_hooks"] = mod
    antenv.axon_hooks = mod
    try:
        sys.path.insert(0, "/root/.axon_site")
        from trn_agent_boot.trn_boot import _ntff_profile_via_ctypes
        hook = _ntff_profile_via_ctypes("/opt/axon/libaxon_pjrt.so")
        if hook is not None:
            mod._hook = hook
    except Exception as e:
        print(f"ntff hook setup failed: {e}")


def timed_run(**inputs):
    """Run once with NTFF tracing; returns exec time in ns (or None)."""
    _ensure_ntff_hook()
    w = _prep_all(inputs)
    wspecs = _weight_specs(w)
    nc = _get_nc("v2", wspecs)
    x1 = np.ascontiguousarray(inputs["x1"], np.float32).reshape(B, C, N)
    x2 = np.ascontiguousarray(inputs["x2"], np.float32).reshape(B, C, N)
    in_maps = [dict(w, x1=x1[b], x2=x2[b]) for b in range(B)]
    res = run_bass_kernel_spmd(nc, in_maps, list(range(NCORES)), trace=True)
    globals()["_LAST_TRACE"] = res
    return res.exec_time_ns


# revision 24
# speedup vs baseline: 1.3664x; 1.3664x over previous
"""Trainium2 Bass kernel for nn_Attention_fusion (sparse_attention fusion block).

Self-contained: takes FULL inputs (B=8 batches), shards batch across 8
NeuronCores (pure data parallel), runs a single fused Bass/Tile kernel per
core, and gathers the full [8,128,128,128] output. BatchNorm batch statistics
are combined across cores with an on-device AllReduce of per-channel moment
partial sums.

v2: pipelined schedule. Phase A (spatial-weight chain + the stats-independent
part of zc) overlaps the input DMA; phase B fuses attention-term/LN/merge/
depthwise-conv/BN-moments into one per-tile loop; engine assignments balance
PE / Act / DVE; c' is spilled to DRAM instead of recomputed; e1 is stored
fp8 (x16 scaling); final affine via diag matmuls + Act bias.
"""
import sys

sys.path.insert(0, "/opt/trn_rl_repo")

import numpy as np

import concourse.bass as bass
import concourse.tile as tile
from concourse import mybir
from concourse.bass_utils import run_bass_kernel_spmd

B, C, H, W = 8, 128, 128, 128
N = H * W
HEADS, HD = 8, 16
EPS_BN = 1e-5
EPS_LN = 1e-5
NCORES = 8
TS = 512                    # free-dim tile size
NT = N // TS                # 32 tiles
PW = W + 2                  # padded row stride (130)
PN = PW * (H + 2)           # padded plane (130*130)
NCH = 8                     # x DMA chunks per input
CHC = N // NCH              # 2048 cols per chunk
DW_SCALE = 16.0             # fp8 scaling for depthwise weights

F32 = mybir.dt.float32
F16 = mybir.dt.float16
F8 = mybir.dt.float8e4
USE_FP8_E1 = False
AX = mybir.AxisListType
ALU = mybir.AluOpType
ACT = mybir.ActivationFunctionType


# ----------------------------------------------------------------------------
# walrus workaround: this container's walrus rejects instructions with more
# than one sync wait command; split extra waits onto standalone EventSemaphore
# instructions on the same engine (program order preserves semantics).
def _split_sync_waits(nc, maxw=1):
    cnt = 0
    for f in nc.m.functions:
        for b in f.blocks:
            insts = b.instructions
            out = []
            changed = False
            for inst in insts:
                si = inst.sync_info
                waits = list(si.on_wait) if si and si.on_wait else []
                if len(waits) > maxw:
                    keep = waits[-maxw:] if maxw > 0 else []
                    extra = waits[: len(waits) - maxw]
                    for wz in extra:
                        es = mybir.InstEventSemaphore(
                            name=f"WSPLIT-{cnt}", ins=[], outs=[]
                        )
                        cnt += 1
                        es.engine = inst.engine
                        es.sync_info = mybir.SyncInfo(on_wait=[wz], on_update=[])
                        out.append(es)
                    inst.sync_info = mybir.SyncInfo(
                        on_wait=keep,
                        on_update=list(si.on_update) if si.on_update else [],
                    )
                    changed = True
                out.append(inst)
            if changed:
                del insts[:]
                insts.extend(out)
    return cnt


# ----------------------------------------------------------------------------
# Host-side weight preparation (identical for every core). All folds:
#  - LN gamma/beta folded into downstream conv weights / biases
#  - mean-centering matrix Cm = I - 11^T/128 folded into the ep weights
#  - attention scale folded into Wk
def _prep_weights(p):
    f32 = lambda a: np.ascontiguousarray(a, np.float32)
    f16 = lambda a: np.ascontiguousarray(a, np.float16)
    w = {}
    J = np.full((C, C), 1.0 / C, np.float64)
    Cm = np.eye(C) - J

    # channel-weights MLP:  y[4C] -> relu(W1 y + b1) -> sigmoid(W2 h + b2)
    W1 = p["cw_w1"].astype(np.float64)      # [512,512]
    W2 = p["cw_w2"].astype(np.float64)      # [256,512]
    w["cw_w1T"] = f16(np.stack([W1[:, k * 128:(k + 1) * 128].T for k in range(4)], 1))
    w["cw_b1"] = f32(p["cw_b1"].reshape(4, 128).T)             # [128,4]
    w["cw_w2T"] = f16(np.stack([W2[:, k * 128:(k + 1) * 128].T for k in range(4)], 1))  # [128,4,256]
    w["cw_b2"] = f32(p["cw_b2"].reshape(2, 128).T)             # [128,2]

    # spatial weight
    w["sw_w1T"] = f16(p["sw_w1"].T)                            # [128,128]
    w["sw_b1"] = f32(p["sw_b1"].reshape(C, 1))
    w["sw_w2T_rep"] = f16(np.repeat(p["sw_w2"].reshape(C, 1), C, 1))  # [128,128]
    w["sw_b2"] = f32(np.full((C, 1), p["sw_b2"][0], np.float32))

    # cross-path projections
    w["cp3T"] = f16(p["cp3_w"].T)
    w["cp3_b"] = f32(p["cp3_b"].reshape(C, 1))
    w["cp4T"] = f16(p["cp4_w"].T)
    w["cp4_b"] = f32(p["cp4_b"].reshape(C, 1))

    # attention: ctxpreT_i = scale * (Wv_i G_i Wk_i^T)  [(h,e),(h,d)]
    scale = HD ** -0.5
    for i, kvw in ((1, p["kv1_w"]), (2, p["kv2_w"])):
        kvw = kvw.astype(np.float64)
        Wk, Wv = kvw[:C], kvw[C:]
        w[f"WkT{i}s"] = f16(Wk.T * scale)                      # [c,(h,d)] scaled
        w[f"WvT{i}"] = f16(Wv.T)                               # [c,(h,e)]

    # ep weights (with Cm centering fold): zc = Cm x + (Cm Ws) s1 + (Cm Wa) a1 + Cm b
    for i, (epw, epb) in ((1, (p["ep1_w"], p["ep1_b"])), (2, (p["ep2_w"], p["ep2_b"]))):
        epw = epw.astype(np.float64)
        Ws, Wa = epw[:, :C], epw[:, C:]
        w[f"WsT{i}c"] = f16((Cm @ Ws).T)                       # [128,128]
        w[f"WaT{i}c"] = f16((Cm @ Wa).T)                       # rhs for device lhsT_a build
        w[f"epb{i}c"] = f32((Cm @ epb.astype(np.float64)).reshape(C, 1))  # [C,1] bias
    w["Cm"] = f16(Cm)                                          # lhsT for the x term (sym)
    w["Jdiv"] = f16(J)                                         # lhsT for var bcast (sym)

    # merge consumers with LN gamma/beta fold
    g1, b1 = p["ln1_g"].astype(np.float64), p["ln1_b"].astype(np.float64)
    g2, b2 = p["ln2_g"].astype(np.float64), p["ln2_b"].astype(np.float64)
    Dres = p["ce_res_w"].astype(np.float64)                    # [128,256]
    Dce1 = p["ce1_w"].astype(np.float64)                       # [128,256]
    w["resT1"] = f16((Dres[:, :C] * g1).T)
    w["resT2"] = f16((Dres[:, C:] * g2).T)
    w["res_bias"] = f32((Dres[:, :C] @ b1 + Dres[:, C:] @ b2).reshape(C, 1))
    w["ce1T1"] = f16((Dce1[:, :C] * g1).T)
    w["ce1T2"] = f16((Dce1[:, C:] * g2).T)
    w["ce1_bias"] = f32((Dce1[:, :C] @ b1 + Dce1[:, C:] @ b2
                         + p["ce1_b"].astype(np.float64)).reshape(C, 1))

    # depthwise 3x3 as 9 diagonal matrices [9,128,128] (tap order dy-major),
    # fp8 with DW_SCALE folded in (undone by the relu activation scale)
    dwk = p["ce_dw_w"].reshape(C, 3, 3).astype(np.float64)
    dwd = np.zeros((9, C, C), np.float64)
    for t in range(9):
        dy, dx = divmod(t, 3)
        np.fill_diagonal(dwd[t], dwk[:, dy, dx] * DW_SCALE)
    w["dw_diag"] = np.ascontiguousarray(
        np.transpose(dwd, (1, 0, 2)), np.float32)              # [C, 9, C] (cast below)
    w["dw_b"] = f32(p["ce_dw_b"].reshape(C, 1))

    w["ce2T"] = f16(p["ce2_w"].T)
    w["ce2_b"] = f32(p["ce2_b"].reshape(C, 1))

    # additive attention mask: 0 on diagonal head blocks, -30000 elsewhere
    m = np.full((C, C), -30000.0, np.float32)
    for hh in range(HEADS):
        m[hh * HD:(hh + 1) * HD, hh * HD:(hh + 1) * HD] = 0.0
    w["attn_mask"] = f32(m)

    # BN params
    w["bn1_g"] = f32(p["ce_bn1_g"].reshape(C, 1))
    w["bn1_b"] = f32(p["ce_bn1_b"].reshape(C, 1))
    w["bn2_g"] = f32(p["ce_bn2_g"].reshape(C, 1))
    w["bn2_b"] = f32(p["ce_bn2_b"].reshape(C, 1))
    return w


def _np_f8(a):
    import ml_dtypes
    return np.ascontiguousarray(np.asarray(a, np.float32).astype(ml_dtypes.float8_e4m3fn))


def _weight_specs(w):
    return {k: (list(v.shape), mybir.dt.from_np(v.dtype)) for k, v in w.items()}


# ----------------------------------------------------------------------------
def _build(wspecs, ncores=NCORES, split=True):
    nc = bass.Bass("TRN2", target_bir_lowering=False, debug=False,
                   num_devices=ncores)
    x1d = nc.dram_tensor("x1", [C, N], F32, kind="ExternalInput").ap()
    x2d = nc.dram_tensor("x2", [C, N], F32, kind="ExternalInput").ap()
    outd = nc.dram_tensor("out", [C, N], F32, kind="ExternalOutput").ap()
    wd = {k: nc.dram_tensor(k, shp, dt, kind="ExternalInput").ap()
          for k, (shp, dt) in wspecs.items()}

    with tile.TileContext(nc) as tc:
        _body(nc, tc, x1d, x2d, outd, wd, ncores)
    if split:
        _split_sync_waits(nc)
    return nc


def _act_rsqrt(nc, out, in_, bias_ap):
    """out = rsqrt(in_ + bias) via the reciprocal_sqrt ACT table (bass's
    activation() refuses Rsqrt; accuracy verified end-to-end)."""
    eng = nc.scalar
    ins = [eng.lower_ap(in_), eng.lower_ap(bias_ap),
           mybir.ImmediateValue(dtype=F32, value=1.0),
           mybir.ImmediateValue(dtype=F32, value=0.0)]
    return eng.add_instruction(
        mybir.InstActivation(
            name=nc.get_next_instruction_name(),
            func=ACT.Rsqrt,
            ins=ins,
            outs=[eng.lower_ap(out)],
        ))


def _body(nc, tc, x1d, x2d, outd, wd, ncores):
    from contextlib import ExitStack
    ctx = ExitStack()
    with ctx:
        wpool = ctx.enter_context(tc.tile_pool(name="w", bufs=1))
        small = ctx.enter_context(tc.tile_pool(name="small", bufs=1))
        dram = ctx.enter_context(tc.tile_pool(name="dram", bufs=1, space="DRAM"))
        # long-lived: zc_partial store (also becomes u store)
        pzc_cm = tc.tile_pool(name="pzc", bufs=1)
        pzc = pzc_cm.__enter__()
        # x store (released after phase 1)
        pxs_cm = tc.tile_pool(name="pxs", bufs=1)
        pxs = pxs_cm.__enter__()
        # trans pools for phase A/1
        trA_cm = tc.tile_pool(name="trA", bufs=2)
        trA = trA_cm.__enter__()
        # PSUM pools: AB spans phase A..B; A1 only phase 1
        psAB_cm = tc.tile_pool(name="psAB", bufs=2, space="PSUM")
        psAB = psAB_cm.__enter__()
        psA1_cm = tc.tile_pool(name="psA1", bufs=2, space="PSUM")
        psA1 = psA1_cm.__enter__()

        # ---- load weights ------------------------------------------------
        wt = {}
        for k in wd:
            shp = list(wd[k].shape)
            if len(shp) == 3:
                flat = [shp[0], shp[1] * shp[2]]
                wt[k] = wpool.tile(flat, wd[k].dtype, tag=k, name="w_" + k)
                nc.sync.dma_start(wt[k][:], wd[k].rearrange("a b c -> a (b c)"))
            else:
                wt[k] = wpool.tile(shp, wd[k].dtype, tag=k, name="w_" + k)
                nc.sync.dma_start(wt[k][:], wd[k][:])

        eps_ln = small.tile([C, 1], F32, tag="eps_ln")
        nc.vector.memset(eps_ln[:], EPS_LN)
        eps_bn = small.tile([C, 1], F32, tag="eps_bn")
        nc.vector.memset(eps_bn[:], EPS_BN)
        # stats accumulators
        avgacc = small.tile([C, 2 * NCH], F32, tag="avgacc")
        m1acc = pxs.tile([C, TS], F16, tag="m1acc")
        m2acc = pxs.tile([C, TS], F16, tag="m2acc")
        nc.vector.memset(m1acc[:], -60000.0)
        nc.vector.memset(m2acc[:], -60000.0)
        # moment stats (bn_stats 6-slot records per tile)
        rstat = small.tile([C, NT * 6], F32, tag="rstat")
        estat = small.tile([C, NT * 6], F32, tag="estat")
        erstat = small.tile([C, NT * 6], F32, tag="erstat")

        xs = [pxs.tile([C, N], F16, tag="x1s", name="x1s"),
              pxs.tile([C, N], F16, tag="x2s", name="x2s")]
        zcp = [pzc.tile([C, N], F16, tag="zcp1", name="zcp1"),
               pzc.tile([C, N], F16, tag="zcp2", name="zcp2")]
        cboth_d = dram.tile([C, 2 * N], F16)

        # ---- phase A + x load, chunk by chunk ----------------------------
        WsTc = [wt["WsT1c"], wt["WsT2c"]]
        epbc = [wt["epb1c"], wt["epb2c"]]
        macc = [m1acc, m2acc]
        # software-pipelined: stage A1(t) / A2(t-1) / A3(t-2) so no engine's
        # in-order queue head-of-line blocks another engine's progress.
        TPC = CHC // TS          # tiles per chunk
        h_ts = {}
        s_ts = {}

        def stA1(t):
            for i in range(2):
                xt = xs[i][:, bass.ts(t, TS)]
                nc.vector.tensor_tensor(macc[i][:], macc[i][:], xt, op=ALU.max)
                ph = psAB.tile([C, TS], F32, tag="T1")
                nc.tensor.matmul(ph[:], wt["sw_w1T"][:], xt, start=True, stop=True)
                h_t = trA.tile([C, TS], F16, tag="h_t", bufs=3, name="h_t")
                nc.vector.tensor_scalar(out=h_t[:], in0=ph[:],
                                        scalar1=wt["sw_b1"][:], scalar2=0.0,
                                        op0=ALU.add, op1=ALU.max)
                h_ts[(t, i)] = h_t

        def stA2(t):
            for i in range(2):
                xt = xs[i][:, bass.ts(t, TS)]
                pl = psAB.tile([C, TS], F32, tag="T1")
                nc.tensor.matmul(pl[:], wt["sw_w2T_rep"][:],
                                 h_ts.pop((t, i))[:], start=True, stop=True)
                sg_t = trA.tile([C, TS], F16, tag="sg_t")
                nc.scalar.activation(sg_t[:], pl[:], ACT.Sigmoid, bias=wt["sw_b2"][:])
                s_t = trA.tile([C, TS], F16, tag="s_t", bufs=3, name="s_t")
                nc.vector.tensor_tensor(s_t[:], sg_t[:], xt, op=ALU.mult)
                s_ts[(t, i)] = s_t

        def stA3(t):
            for i in range(2):
                xt = xs[i][:, bass.ts(t, TS)]
                pz = psAB.tile([C, TS], F32, tag="T2")
                nc.tensor.matmul(pz[:], WsTc[i][:], s_ts.pop((t, i))[:],
                                 start=True, stop=False)
                nc.tensor.matmul(pz[:], wt["Cm"][:], xt, start=False, stop=True)
                if i == 0:
                    nc.vector.tensor_scalar(out=zcp[i][:, bass.ts(t, TS)],
                                            in0=pz[:], scalar1=epbc[i][:],
                                            scalar2=None, op0=ALU.add)
                else:
                    nc.scalar.activation(zcp[i][:, bass.ts(t, TS)], pz[:],
                                         ACT.Identity, bias=epbc[i][:])

        for tt in range(NT + 2):
            if tt < NT and tt % TPC == 0:
                c_ = tt // TPC
                csl = bass.ts(c_, CHC)
                for i, xd in ((0, x1d), (1, x2d)):
                    nc.gpsimd.dma_start(xs[i][:, csl], xd[:, csl])
                for i in range(2):
                    junk = trA.tile([C, CHC], F16, tag="sjunk", bufs=1)
                    nc.scalar.activation(
                        junk[:], xs[i][:, csl], ACT.Copy,
                        accum_out=avgacc[:, i * NCH + c_:i * NCH + c_ + 1])
            if tt < NT:
                stA1(tt)
            if 1 <= tt <= NT:
                stA2(tt - 1)
            if tt >= 2:
                stA3(tt - 2)

        # ---- stats -> channel MLP ---------------------------------------
        stat_y = small.tile([C, 4], F32, tag="stat_y")       # avg1 avg2 mx1 mx2
        for i in range(2):
            nc.vector.tensor_reduce(
                out=stat_y[:, i:i + 1],
                in_=avgacc[:, i * NCH:(i + 1) * NCH],
                axis=AX.X, op=ALU.add)
            mh = small.tile([C, 256], F16, tag=f"mh{i}")
            nc.vector.tensor_tensor(mh[:], macc[i][:, :256], macc[i][:, 256:], op=ALU.max)
            mq = small.tile([C, 128], F16, tag=f"mq{i}")
            nc.vector.tensor_tensor(mq[:], mh[:, :128], mh[:, 128:], op=ALU.max)
            nc.vector.tensor_reduce(out=stat_y[:, 2 + i:3 + i], in_=mq[:],
                                    axis=AX.X, op=ALU.max)
        y16 = small.tile([C, 4], F16, tag="y16")
        nc.vector.tensor_scalar(out=y16[:, 0:2], in0=stat_y[:, 0:2],
                                scalar1=1.0 / N, scalar2=None, op0=ALU.mult)
        nc.vector.tensor_copy(y16[:, 2:4], stat_y[:, 2:4])

        h16 = small.tile([C, 4], F16, tag="h16")
        for oc in range(4):
            ph = psAB.tile([C, TS], F32, tag="T1")
            for kc in range(4):
                nc.tensor.matmul(
                    ph[:, :1], wt["cw_w1T"][:, kc * 512 + oc * 128: kc * 512 + (oc + 1) * 128],
                    y16[:, kc:kc + 1], start=(kc == 0), stop=(kc == 3))
            nc.scalar.activation(h16[:, oc:oc + 1], ph[:, :1], ACT.Relu,
                                 bias=wt["cw_b1"][:, oc:oc + 1])
        cw = small.tile([C, 2], F32, tag="cw")
        for oc in range(2):
            ph = psAB.tile([C, TS], F32, tag="T1")
            for kc in range(4):
                nc.tensor.matmul(
                    ph[:, :1], wt["cw_w2T"][:, kc * 256 + oc * 128: kc * 256 + (oc + 1) * 128],
                    h16[:, kc:kc + 1], start=(kc == 0), stop=(kc == 3))
            nc.scalar.activation(cw[:, oc:oc + 1], ph[:, :1], ACT.Sigmoid,
                                 bias=wt["cw_b2"][:, oc:oc + 1])
        cpTc = []
        for i in range(2):
            cc = small.tile([C, C], F16, tag=f"cpTc_{i}")
            nc.vector.tensor_scalar(out=cc[:], in0=wt[("cp3T", "cp4T")[i]][:],
                                    scalar1=cw[:, i:i + 1], scalar2=None,
                                    op0=ALU.mult)
            cpTc.append(cc)

        # ---- phase 1: cp chain + Gram (c' spilled to DRAM) ---------------
        # staggered: cp(t) / transpose(t-1) / gram(t-2)
        gp = psA1.tile([C, 2 * C], F32, tag="gram", bufs=1, name="gram")
        cpB = [wt["cp3_b"], wt["cp4_b"]]
        cpairs = {}
        ctts = {}

        def st1cp(t):
            cpair = trA.tile([C, 2 * TS], F16, tag="cpair", bufs=3, name="cpair")
            for i in range(2):
                pc = psAB.tile([C, TS], F32, tag="T1")
                nc.tensor.matmul(pc[:], cpTc[i][:], xs[i][:, bass.ts(t, TS)],
                                 start=True, stop=True)
                cdst = cpair[:, i * TS:(i + 1) * TS]
                if i == 0:
                    nc.vector.tensor_scalar(out=cdst, in0=pc[:],
                                            scalar1=cpB[i][:], scalar2=0.0,
                                            op0=ALU.add, op1=ALU.max)
                else:
                    nc.scalar.activation(cdst, pc[:], ACT.Relu, bias=cpB[i][:])
            nc.gpsimd.dma_start(cboth_d[:, t * 2 * TS:(t + 1) * 2 * TS], cpair[:])
            cpairs[t] = cpair

        def st1tr(t):
            cpair = cpairs.pop(t)
            cts = []
            for i in range(2):
                pt = psA1.tile([C, TS], F16, tag="pt", bufs=2)
                for j in range(4):
                    nc.tensor.transpose(
                        pt[:, j * 128:(j + 1) * 128],
                        cpair[:, i * TS + j * 128: i * TS + (j + 1) * 128],
                        wt["ident"][:])
                ctt = trA.tile([C, TS], F16, tag=f"ctt{i}", bufs=2, name="ctt")
                if i == 0:
                    nc.vector.tensor_copy(ctt[:], pt[:])
                else:
                    nc.scalar.copy(ctt[:], pt[:])
                cts.append(ctt)
            ctts[t] = cts

        def st1gram(t):
            cts = ctts.pop(t)
            for i in range(2):
                for j in range(4):
                    nc.tensor.matmul(gp[:, i * C:(i + 1) * C],
                                     cts[i][:, j * 128:(j + 1) * 128],
                                     cts[i][:, j * 128:(j + 1) * 128],
                                     start=(t == 0 and j == 0),
                                     stop=(t == NT - 1 and j == 3))

        for tt in range(NT + 2):
            if tt < NT:
                st1cp(tt)
            if 1 <= tt <= NT:
                st1tr(tt - 1)
            if tt >= 2:
                st1gram(tt - 2)

        # ---- attention context smalls ------------------------------------
        bdp = []
        for i in range(2):
            g16 = small.tile([C, C], F16, tag=f"g16_{i}")
            nc.vector.tensor_copy(g16[:], gp[:, i * C:(i + 1) * C])
            pm = psAB.tile([C, TS], F32, tag="T1")
            nc.tensor.matmul(pm[:, :C], g16[:], wt[f"WkT{i+1}s"][:], start=True, stop=True)
            m16 = small.tile([C, C], F16, tag=f"m16_{i}")
            nc.vector.tensor_copy(m16[:], pm[:, :C])
            pc2 = psAB.tile([C, TS], F32, tag="T1")
            nc.tensor.matmul(pc2[:, :C], wt[f"WvT{i+1}"][:], m16[:], start=True, stop=True)
            cm_t = small.tile([C, C], F32, tag=f"cm_{i}")
            nc.vector.tensor_tensor(cm_t[:], pc2[:, :C], wt["attn_mask"][:], op=ALU.add)
            negmx = small.tile([C, 1], F32, tag=f"negmx_{i}")
            nc.vector.tensor_reduce(out=negmx[:], in_=cm_t[:], axis=AX.X,
                                    op=ALU.max, negate=True)
            ex = small.tile([C, C], F32, tag=f"ex_{i}")
            nc.scalar.activation(ex[:], cm_t[:], ACT.Exp, bias=negmx[:])
            sm = small.tile([C, 1], F32, tag=f"sm_{i}")
            nc.vector.tensor_reduce(out=sm[:], in_=ex[:], axis=AX.X, op=ALU.add)
            rs = small.tile([C, 1], F32, tag=f"rs_{i}")
            nc.vector.reciprocal(rs[:], sm[:])
            bd = small.tile([C, C], F16, tag=f"bd_{i}")
            nc.vector.tensor_scalar(out=bd[:], in0=ex[:], scalar1=rs[:],
                                    scalar2=None, op0=ALU.mult)
            bdp.append(bd)
        lhsTa = []
        for i in range(2):
            pa = psAB.tile([C, TS], F32, tag="T1")
            nc.tensor.matmul(pa[:, :C], bdp[1 - i][:], wt[f"WaT{i+1}c"][:],
                             start=True, stop=True)
            a16 = small.tile([C, C], F16, tag=f"a16_{i}")
            nc.vector.tensor_copy(a16[:], pa[:, :C])
            lhsTa.append(a16)

        # ---- close phase A/1 pools, open phase B pools -------------------
        psA1_cm.__exit__(None, None, None)
        trA_cm.__exit__(None, None, None)
        pxs_cm.__exit__(None, None, None)
        psB_cm = tc.tile_pool(name="psB", bufs=2, space="PSUM")
        psB = psB_cm.__enter__()
        pconv_cm = tc.tile_pool(name="pconv", bufs=1)
        pconv = pconv_cm.__enter__()
        trB_cm = tc.tile_pool(name="trB", bufs=2)
        trB = trB_cm.__enter__()

        res_s = pconv.tile([C, N], F16, tag="res_s")
        e_s = pconv.tile([C, N], F16, tag="e_s")
        e1p = pconv.tile([C, PN], F8 if USE_FP8_E1 else F16, tag="e1p")
        nc.vector.memset(e1p[:], 0.0)   # border stays zero
        e1v = e1p[:].rearrange("p (h w) -> p h w", w=PW)

        # ---- phase B: software-pipelined stages --------------------------
        # S1(t): attention-term matmuls + zc assembly + square   (PE, DVE)
        # S4(t-1): LN variance + rstd + u                        (PE, Act, DVE)
        # S7(t-2): res / e1 matmuls + evacs + res stats          (PE, Act, DVE)
        # S9(t-4): depthwise conv + ce2 + e/er stats             (PE, Act, DVE)
        crls, zcprs, zqprs = {}, {}, {}

        def c_reload(t):
            crl = trB.tile([C, 2 * TS], F16, tag="crl", bufs=3, name="crl")
            nc.gpsimd.dma_start(crl[:], cboth_d[:, t * 2 * TS:(t + 1) * 2 * TS])
            crls[t] = crl

        def stB1(t):
            sl = bass.ts(t, TS)
            crl = crls.pop(t)
            zcpr = trB.tile([C, 2 * TS], F16, tag="zcpr", bufs=2, name="zcpr")
            for i in range(2):
                pz = psAB.tile([C, TS], F32, tag="T2")
                nc.tensor.matmul(pz[:], lhsTa[i][:], crl[:, i * TS:(i + 1) * TS],
                                 start=True, stop=True)
                nc.vector.tensor_tensor(zcpr[:, i * TS:(i + 1) * TS], pz[:],
                                        zcp[i][:, sl], op=ALU.add)
            zqpr = trB.tile([C, 2 * TS], F16, tag="zqpr", bufs=2, name="zqpr")
            nc.vector.tensor_tensor(zqpr[:], zcpr[:], zcpr[:], op=ALU.mult)
            zcprs[t] = zcpr
            zqprs[t] = zqpr

        def stB4(t):
            sl = bass.ts(t, TS)
            zcpr = zcprs.pop(t)
            zqpr = zqprs.pop(t)
            rstdp = trB.tile([C, 2 * TS], F16, tag="rstdp", bufs=2, name="rstdp")
            for i in range(2):
                pv = psAB.tile([C, TS], F32, tag="T1")
                nc.tensor.matmul(pv[:], wt["Jdiv"][:], zqpr[:, i * TS:(i + 1) * TS],
                                 start=True, stop=True)
                _act_rsqrt(nc, rstdp[:, i * TS:(i + 1) * TS], pv[:], eps_ln[:])
            for i in range(2):
                # u overwrites the zc_partial store
                nc.vector.tensor_tensor(zcp[i][:, sl], zcpr[:, i * TS:(i + 1) * TS],
                                        rstdp[:, i * TS:(i + 1) * TS], op=ALU.mult)

        def stB7(t):
            sl = bass.ts(t, TS)
            u = [zcp[0][:, sl], zcp[1][:, sl]]
            pr = psAB.tile([C, TS], F32, tag="T2")
            nc.tensor.matmul(pr[:], wt["resT1"][:], u[0], start=True, stop=False)
            nc.tensor.matmul(pr[:], wt["resT2"][:], u[1], start=False, stop=True)
            nc.scalar.activation(res_s[:, sl], pr[:], ACT.Identity,
                                 bias=wt["res_bias"][:])
            nc.vector.bn_stats(rstat[:, t * 6:(t + 1) * 6], res_s[:, sl])
            pe1 = psB.tile([C, TS], F32, tag="e1ps")
            nc.tensor.matmul(pe1[:], wt["ce1T1"][:], u[0], start=True, stop=False)
            nc.tensor.matmul(pe1[:], wt["ce1T2"][:], u[1], start=False, stop=True)
            h0 = t * 4
            nc.scalar.activation(
                e1v[:, h0 + 1: h0 + 5, 1: 1 + W],
                pe1[:].rearrange("p (a b) -> p a b", b=W),
                ACT.Identity, bias=wt["ce1_bias"][:])

        def stB9(t):
            sl = bass.ts(t, TS)
            h0 = t * 4
            pdw = psB.tile([C, TS], F32, tag="dw")
            for tap in range(9):
                dy, dx = divmod(tap, 3)
                rhs = e1v[:, h0 + dy: h0 + dy + 4, dx: dx + W]
                nc.tensor.matmul(pdw[:], wt["dw_diag"][:, tap * C:(tap + 1) * C],
                                 rhs, start=(tap == 0), stop=(tap == 8))
            e2_t = trB.tile([C, TS], F16, tag="e2_t", bufs=1)
            nc.scalar.activation(e2_t[:], pdw[:], ACT.Relu, bias=wt["dw_b"][:],
                                 scale=1.0 / DW_SCALE)
            pce = psAB.tile([C, TS], F32, tag="T2")
            nc.tensor.matmul(pce[:], wt["ce2T"][:], e2_t[:], start=True, stop=True)
            nc.scalar.activation(e_s[:, sl], pce[:], ACT.Identity,
                                 bias=wt["ce2_b"][:])
            nc.vector.bn_stats(estat[:, t * 6:(t + 1) * 6], e_s[:, sl])
            j2 = trB.tile([C, TS], F16, tag="mjunk", bufs=1)
            nc.vector.tensor_tensor(j2[:], e_s[:, sl], res_s[:, sl], op=ALU.mult)
            nc.vector.bn_stats(erstat[:, t * 6:(t + 1) * 6], j2[:])

        c_reload(0)
        c_reload(1)
        for tt in range(NT + 4):
            if tt + 2 < NT:
                c_reload(tt + 2)
            if tt < NT:
                stB1(tt)
            if 1 <= tt <= NT:
                stB4(tt - 1)
            if 2 <= tt <= NT + 1:
                stB7(tt - 2)
            if tt >= 4:
                stB9(tt - 4)

        # ---- cross-core moment allreduce + coefficients ------------------
        parts = small.tile([C, 8], F32, tag="parts")
        nc.vector.memset(parts[:], 0.0)
        HN = TS // 2  # elements per even/odd half

        def agg_stats(stat, sum_col, sq_col):
            """parts[sum_col] = sum(x); parts[sq_col] = sum(x^2) (if not None),
            reconstructed from per-tile bn_stats records."""
            v = stat[:].rearrange("p (t k) -> p t k", k=6)
            wrk = small.tile([C, NT], F32, tag="aggw1", uniquify=True)
            wv = wrk[:].rearrange("p (t k) -> p t k", k=1)
            nc.vector.tensor_tensor(wv[:], v[:, :, 1:2], v[:, :, 4:5], op=ALU.add)
            nc.vector.tensor_reduce(out=parts[:, sum_col:sum_col + 1], in_=wrk[:],
                                    axis=AX.X, op=ALU.add)
            nc.vector.tensor_scalar(out=parts[:, sum_col:sum_col + 1],
                                    in0=parts[:, sum_col:sum_col + 1],
                                    scalar1=float(HN), scalar2=None, op0=ALU.mult)
            if sq_col is None:
                return
            w2 = small.tile([C, NT], F32, tag="aggw2", uniquify=True)
            w2v = w2[:].rearrange("p (t k) -> p t k", k=1)
            w3 = small.tile([C, NT], F32, tag="aggw3", uniquify=True)
            w3v = w3[:].rearrange("p (t k) -> p t k", k=1)
            # w2 = HN*(me^2 + mo^2) + M2e + M2o, per tile
            nc.vector.tensor_tensor(w2v[:], v[:, :, 1:2], v[:, :, 1:2], op=ALU.mult)
            nc.vector.tensor_tensor(w3v[:], v[:, :, 4:5], v[:, :, 4:5], op=ALU.mult)
            nc.vector.tensor_tensor(w2[:], w2[:], w3[:], op=ALU.add)
            nc.vector.tensor_scalar(out=w2[:], in0=w2[:], scalar1=float(HN),
                                    scalar2=None, op0=ALU.mult)
            nc.vector.tensor_tensor(w3v[:], v[:, :, 2:3], v[:, :, 5:6], op=ALU.add)
            nc.vector.tensor_tensor(w2[:], w2[:], w3[:], op=ALU.add)
            nc.vector.tensor_reduce(out=parts[:, sq_col:sq_col + 1], in_=w2[:],
                                    axis=AX.X, op=ALU.add)

        agg_stats(estat, 0, 1)
        agg_stats(rstat, 2, 3)
        agg_stats(erstat, 4, None)
        cin = dram.tile([C, 8], F32)
        cout = dram.tile([C, 8], F32)
        nc.sync.dma_start(cin[:], parts[:])
        nc.gpsimd.collective_compute(
            "AllReduce", ALU.add,
            replica_groups=[list(range(ncores))],
            ins=[cin.opt()], outs=[cout.opt()])
        allr = small.tile([C, 8], F32, tag="allr")
        nc.sync.dma_start(allr[:], cout[:])

        # per-channel BN coefficient math (tiny [128,1] ops)
        NTOT = float(ncores * N)
        cf = small.tile([C, 12], F32, tag="cf")
        nc.vector.tensor_scalar(out=cf[:, 0:1], in0=allr[:, 0:1], scalar1=1.0 / NTOT,
                                scalar2=None, op0=ALU.mult)               # me
        nc.vector.tensor_scalar(out=cf[:, 1:2], in0=allr[:, 1:2], scalar1=1.0 / NTOT,
                                scalar2=None, op0=ALU.mult)               # E e2
        nc.vector.tensor_tensor(cf[:, 2:3], cf[:, 0:1], cf[:, 0:1], op=ALU.mult)
        nc.vector.tensor_tensor(cf[:, 2:3], cf[:, 1:2], cf[:, 2:3], op=ALU.subtract)  # ve
        tmp = small.tile([C, 4], F32, tag="cftmp")
        nc.scalar.activation(tmp[:, 0:1], cf[:, 2:3], ACT.Sqrt, bias=eps_bn[:])
        nc.vector.reciprocal(tmp[:, 1:2], tmp[:, 0:1])                     # rsq_e
        nc.vector.tensor_tensor(cf[:, 3:4], wt["bn1_g"][:], tmp[:, 1:2], op=ALU.mult)  # a1
        nc.vector.tensor_tensor(cf[:, 4:5], cf[:, 0:1], cf[:, 3:4], op=ALU.mult)
        nc.vector.tensor_tensor(cf[:, 4:5], wt["bn1_b"][:], cf[:, 4:5], op=ALU.subtract)  # b1f
        nc.vector.tensor_tensor(cf[:, 5:6], cf[:, 3:4], allr[:, 0:1], op=ALU.mult)
        nc.vector.tensor_tensor(cf[:, 5:6], allr[:, 2:3], cf[:, 5:6], op=ALU.add)
        nc.vector.tensor_scalar(out=cf[:, 5:6], in0=cf[:, 5:6], scalar1=1.0 / NTOT,
                                scalar2=None, op0=ALU.mult)
        nc.vector.tensor_tensor(cf[:, 5:6], cf[:, 5:6], cf[:, 4:5], op=ALU.add)   # ms
        nc.vector.tensor_tensor(tmp[:, 2:3], cf[:, 3:4], cf[:, 3:4], op=ALU.mult)  # a1^2
        nc.vector.tensor_tensor(tmp[:, 3:4], tmp[:, 2:3], allr[:, 1:2], op=ALU.mult)
        nc.vector.tensor_tensor(cf[:, 6:7], cf[:, 3:4], allr[:, 4:5], op=ALU.mult)
        nc.vector.tensor_scalar(out=cf[:, 6:7], in0=cf[:, 6:7], scalar1=2.0,
                                scalar2=None, op0=ALU.mult)
        nc.vector.tensor_tensor(cf[:, 6:7], cf[:, 6:7], tmp[:, 3:4], op=ALU.add)
        nc.vector.tensor_tensor(cf[:, 6:7], cf[:, 6:7], allr[:, 3:4], op=ALU.add)
        nc.vector.tensor_scalar(out=cf[:, 6:7], in0=cf[:, 6:7], scalar1=1.0 / NTOT,
                                scalar2=None, op0=ALU.mult)
        nc.vector.tensor_tensor(tmp[:, 2:3], cf[:, 5:6], cf[:, 4:5], op=ALU.subtract)
        nc.vector.tensor_tensor(tmp[:, 2:3], tmp[:, 2:3], cf[:, 4:5], op=ALU.mult)
        nc.vector.tensor_scalar(out=tmp[:, 2:3], in0=tmp[:, 2:3], scalar1=2.0,
                                scalar2=None, op0=ALU.mult)
        nc.vector.tensor_tensor(cf[:, 6:7], cf[:, 6:7], tmp[:, 2:3], op=ALU.add)
        nc.vector.tensor_tensor(tmp[:, 2:3], cf[:, 4:5], cf[:, 4:5], op=ALU.mult)
        nc.vector.tensor_tensor(cf[:, 6:7], cf[:, 6:7], tmp[:, 2:3], op=ALU.add)  # Es2
        nc.vector.tensor_tensor(tmp[:, 2:3], cf[:, 5:6], cf[:, 5:6], op=ALU.mult)
        nc.vector.tensor_tensor(cf[:, 7:8], cf[:, 6:7], tmp[:, 2:3], op=ALU.subtract)  # vs
        nc.scalar.activation(tmp[:, 0:1], cf[:, 7:8], ACT.Sqrt, bias=eps_bn[:])
        nc.vector.reciprocal(tmp[:, 1:2], tmp[:, 0:1])
        nc.vector.tensor_tensor(cf[:, 8:9], wt["bn2_g"][:], tmp[:, 1:2], op=ALU.mult)  # a2
        nc.vector.tensor_tensor(cf[:, 9:10], cf[:, 8:9], cf[:, 3:4], op=ALU.mult)      # ae
        # c0 = b2 - a2*(ms - b1f)
        nc.vector.tensor_tensor(tmp[:, 2:3], cf[:, 5:6], cf[:, 4:5], op=ALU.subtract)
        nc.vector.tensor_tensor(tmp[:, 2:3], tmp[:, 2:3], cf[:, 8:9], op=ALU.mult)
        nc.vector.tensor_tensor(cf[:, 10:11], wt["bn2_b"][:], tmp[:, 2:3],
                                op=ALU.subtract)
        # diag matrices for the final affine
        dae = small.tile([C, C], F16, tag="dae")
        nc.vector.tensor_scalar(out=dae[:], in0=wt["ident"][:],
                                scalar1=cf[:, 9:10], scalar2=None, op0=ALU.mult)
        da2 = small.tile([C, C], F16, tag="da2")
        nc.vector.tensor_scalar(out=da2[:], in0=wt["ident"][:],
                                scalar1=cf[:, 8:9], scalar2=None, op0=ALU.mult)

        # ---- final affine + store (staggered mm / evac / DMA) ------------
        psB_cm.__exit__(None, None, None)
        psF_cm = tc.tile_pool(name="psF", bufs=4, space="PSUM")
        psF = psF_cm.__enter__()
        pfs, outts = {}, {}

        def stFmm(t):
            pf = psF.tile([C, TS], F32, tag="fin", name="pf")
            nc.tensor.matmul(pf[:], dae[:], e_s[:, bass.ts(t, TS)],
                             start=True, stop=False)
            nc.tensor.matmul(pf[:], da2[:], res_s[:, bass.ts(t, TS)],
                             start=False, stop=True)
            pfs[t] = pf

        def stFev(t):
            out_t = trB.tile([C, TS], F16, tag="out_t", bufs=2, name="out_t")
            nc.scalar.activation(out_t[:], pfs.pop(t)[:], ACT.Identity,
                                 bias=cf[:, 10:11])
            outts[t] = out_t

        for tt in range(NT + 2):
            if tt < NT:
                stFmm(tt)
            if 1 <= tt <= NT:
                stFev(tt - 1)
            if tt >= 2:
                nc.gpsimd.dma_start(outd[:, bass.ts(tt - 2, TS)], outts.pop(tt - 2)[:])
        psF_cm.__exit__(None, None, None)
        trB_cm.__exit__(None, None, None)
        pconv_cm.__exit__(None, None, None)
        psAB_cm.__exit__(None, None, None)
        pzc_cm.__exit__(None, None, None)


_CACHE = {}


def _get_nc(wspecs_key, wspecs, ncores=NCORES):
    key = (wspecs_key, ncores)
    if key not in _CACHE:
        _CACHE[key] = _build(wspecs, ncores=ncores)
    return _CACHE[key]


def _prep_all(inputs):
    w = _prep_weights(inputs)
    w["dw_diag"] = _np_f8(w["dw_diag"]) if USE_FP8_E1 else \
        np.ascontiguousarray(w["dw_diag"], np.float16)
    w["ident"] = np.eye(C, dtype=np.float16)
    return w


def kernel(**inputs):
    w = _prep_all(inputs)
    wspecs = _weight_specs(w)
    nc = _get_nc("v2", wspecs)
    x1 = np.ascontiguousarray(inputs["x1"], np.float32).reshape(B, C, N)
    x2 = np.ascontiguousarray(inputs["x2"], np.float32).reshape(B, C, N)
    in_maps = [dict(w, x1=x1[b], x2=x2[b]) for b in range(B)]
    res = run_bass_kernel_spmd(nc, in_maps, list(range(NCORES)))
    out = np.stack([res.results[b]["out"].reshape(C, H, W) for b in range(B)])
    return out.astype(np.float32)


def _ensure_ntff_hook():
    """Synthesize antenv.axon_hooks (missing in this image) and register the
    ctypes NTFF profile hook against the axon PJRT .so."""
    import types
    import antenv
    if getattr(antenv, "axon_hooks", None) is not None:
        return
    mod = types.ModuleType("antenv.axon_hooks")
    mod._hook = None
    def set_axon_ntff_profile_hook(h):
        mod._hook = h
    def get_axon_ntff_profile_hook():
        return mod._hook
    mod.set_axon_ntff_profile_hook = set_axon_ntff_profile_hook
    mod.get_axon_ntff_profile_hook = get_axon_ntff_profile_hook
    sys.modules["antenv.axon_hooks"] = mod
    antenv.axon_hooks = mod
    try:
        sys.path.insert(0, "/root/.axon_site")
        from trn_agent_boot.trn_boot import _ntff_profile_via_ctypes
        hook = _ntff_profile_via_ctypes("/opt/axon/libaxon_pjrt.so")
        if hook is not None:
            mod._hook = hook
    except Exception as e:
        print(f"ntff hook setup failed: {e}")


def timed_run(**inputs):
    """Run once with NTFF tracing; returns exec time in ns (or None)."""
    _ensure_ntff_hook()
    w = _prep_all(inputs)
    wspecs = _weight_specs(w)
    nc = _get_nc("v2", wspecs)
    x1 = np.ascontiguousarray(inputs["x1"], np.float32).reshape(B, C, N)
    x2 = np.ascontiguousarray(inputs["x2"], np.float32).reshape(B, C, N)
    in_maps = [dict(w, x1=x1[b], x2=x2[b]) for b in range(B)]
    res = run_bass_kernel_spmd(nc, in_maps, list(range(NCORES)), trace=True)
    globals()["_LAST_TRACE"] = res
    return res.exec_time_ns
